# revision 6
# baseline (speedup 1.0000x reference)
"""NNUE-MCTS model kernel for 8 Trainium2 NeuronCores.

Strategy (pure data parallel, batch sharded 8 ways; each core runs the same
Bass program on its 32768-row shard):

  Per core, transposed pipeline (feature dims on SBUF partitions, batch on
  the free dim), in superblocks of 512 batch columns:

  1. Selection logic in natural layout (batch on partitions), vectorized over
     mega-tiles of 32x128 batch rows: masks -> segmented cumsum (one
     tensor_tensor_scan) -> top-3 selected feature ids per side (rank r value
     falls out as sum(X * mask * [incl==r+1]) which is also 0 == pad id when
     fewer than 3) -> duplicate merge into (index, count) pairs with killed
     slots pushed to negative indices.
  2. GPSIMD local_scatter builds the two-sided one-hot count matrix
     A (128 batch, 4 tiles x [stm 128 | nstm 128]) in fp16.
  3. DMA xbar transpose (2-byte) flips each 128x128 block -> A^T with feature
     on partitions, batch on free: directly streamable by the PE.
  4. FT embedding = matmul(ft_w_chunk (128,128) stationary, A^T moving) into
     PSUM; relu(+ft_b) moves PSUM->SBUF fp16 split across DVE and ACT.
  5. W1 (5 K-chunks, dense chunk carries a const-1 row that also produces the
     h const row for bias folding), relu -> h (33, 512) fp16.
  6. W2p/W2v packed into one (33, 61) matmul; b2 folded via h's const row.
     Policy rows DMA out transposed; value row staged to DRAM, tanh'd in one
     (64, 512) pass at the end.

Everything the PE touches is fp16 (exact for the one-hot counts / indices;
~1e-3 relative for weights/activations); PSUM accumulation is fp32.
"""

import numpy as np

B_TOTAL = 262144
N_CORES = 8
BS = B_TOTAL // N_CORES          # 32768 rows per core
NT = BS // 128                   # 256 tiles of 128 rows
MEGA = 32                        # tiles per selection mega-block
NMEGA = NT // MEGA               # 8
SBT = 4                          # tiles per superblock
NSB = NT // SBT                  # 64 superblocks
BT = 128 * SBT                   # 512 batch columns per superblock
SB_PER_MEGA = MEGA // SBT        # 8

FT_DIM = 256
PIECE = 120
P1_CUT = 60
DENSE = 66
HID = 32
HEX = 60
KILL = -8192.0                   # pushes a duplicate slot's index negative

_CACHE = {}


def _build_nc():
    import concourse.bacc as bacc
    import concourse.mybir as mybir
    from concourse.tile import TileContext

    dt = mybir.dt
    alu = mybir.AluOpType
    AF = mybir.ActivationFunctionType
    f16, f32, i16 = dt.float16, dt.float32, dt.int16

    nc = bacc.Bacc("TRN2", target_bir_lowering=False, debug=False)

    # ---- DRAM I/O (per-core shapes) ----
    d_sp = nc.dram_tensor("x_sparse", (NMEGA, 128, MEGA * 6), f16, kind="ExternalInput")
    d_st = nc.dram_tensor("x_stm6", (NMEGA, 128, MEGA * 6), f16, kind="ExternalInput")
    d_de = nc.dram_tensor("x_dense", (DENSE + 1, BS), f16, kind="ExternalInput")
    d_cj = nc.dram_tensor("c_j", (128, MEGA * 6), f16, kind="ExternalInput")
    d_cr = nc.dram_tensor("c_r", (128, MEGA * 6), f16, kind="ExternalInput")
    d_co = nc.dram_tensor("c_o", (128, MEGA * 6), f16, kind="ExternalInput")
    d_wft = nc.dram_tensor("w_ft", (128, FT_DIM), f16, kind="ExternalInput")
    d_ftb = nc.dram_tensor("w_ftb", (128, 2), f32, kind="ExternalInput")
    d_w1c = nc.dram_tensor("w_w1c", (128, 128), f16, kind="ExternalInput")
    d_w1d = nc.dram_tensor("w_w1d", (DENSE + 1, HID + 1), f16, kind="ExternalInput")
    d_w2 = nc.dram_tensor("w_w2", (HID + 1, HEX + 1), f16, kind="ExternalInput")
    d_pol = nc.dram_tensor("policy_t", (HEX, BS), f32, kind="ExternalOutput")
    d_val = nc.dram_tensor("value_s", (NSB, BT), f32, kind="ExternalOutput")

    ap_sp, ap_st, ap_de = d_sp.ap(), d_st.ap(), d_de.ap()
    ap_pol, ap_val = d_pol.ap(), d_val.ap()

    with TileContext(nc) as tc:
        with tc.tile_pool(name="const", bufs=1) as cpool, \
             tc.tile_pool(name="sel", bufs=2) as spool, \
             tc.tile_pool(name="mm", bufs=3) as mpool, \
             tc.tile_pool(name="dstage", bufs=1, space="DRAM") as dpool, \
             tc.tile_pool(name="ps", bufs=1, space="PSUM") as ppool:

            # ---- constants / weights into SBUF once ----
            wft = cpool.tile([128, FT_DIM], f16)
            nc.sync.dma_start(out=wft[:], in_=d_wft.ap()[:])
            ftb = cpool.tile([128, 2], f32)
            nc.sync.dma_start(out=ftb[:], in_=d_ftb.ap()[:])
            w1c = cpool.tile([128, 128], f16)
            nc.sync.dma_start(out=w1c[:], in_=d_w1c.ap()[:])
            w1d = cpool.tile([DENSE + 1, HID + 1], f16)
            nc.sync.dma_start(out=w1d[:], in_=d_w1d.ap()[:])
            w2 = cpool.tile([HID + 1, HEX + 1], f16)
            nc.sync.dma_start(out=w2[:], in_=d_w2.ap()[:])
            cj = cpool.tile([128, MEGA * 6], f16)
            nc.sync.dma_start(out=cj[:], in_=d_cj.ap()[:])
            cr = cpool.tile([128, MEGA * 6], f16)
            nc.sync.dma_start(out=cr[:], in_=d_cr.ap()[:])
            co = cpool.tile([128, MEGA * 6], f16)
            nc.sync.dma_start(out=co[:], in_=d_co.ap()[:])

            vstage = dpool.tile([NSB, BT], f32)

            for mg in range(NMEGA):
                # ================= selection (natural layout) =================
                X = spool.tile([128, MEGA * 6], f16)
                nc.sync.dma_start(out=X[:], in_=ap_sp[mg])
                T6 = spool.tile([128, MEGA * 6], f16)
                nc.sync.dma_start(out=T6[:], in_=ap_st[mg])

                p1 = spool.tile([128, MEGA * 6], f16)
                nc.vector.tensor_scalar(out=p1[:], in0=X[:], scalar1=float(P1_CUT),
                                        scalar2=None, op0=alu.is_lt)
                msk = spool.tile([128, MEGA * 6], f16)
                nc.vector.tensor_tensor(out=msk[:], in0=p1[:], in1=T6[:],
                                        op=alu.is_equal)
                # segmented inclusive prefix count: state = cr*state + msk
                incl = spool.tile([128, MEGA * 6], f16)
                nc.vector.tensor_tensor_scan(out=incl[:], data0=cr[:], data1=msk[:],
                                             initial=0.0, op0=alu.mult, op1=alu.add)
                incn = spool.tile([128, MEGA * 6], f16)
                nc.vector.tensor_tensor(out=incn[:], in0=cj[:], in1=incl[:],
                                        op=alu.subtract)
                ws = spool.tile([128, MEGA * 6], f16)
                nc.vector.tensor_tensor(out=ws[:], in0=X[:], in1=msk[:], op=alu.mult)
                wn = spool.tile([128, MEGA * 6], f16)
                nc.vector.tensor_tensor(out=wn[:], in0=X[:], in1=ws[:],
                                        op=alu.subtract)

                # P tiles, viewed (128, MEGA, 6): slots 0..2 stm, 3..5 nstm
                Ps = spool.tile([128, MEGA * 6], f16)
                Ps3 = Ps.rearrange("p (t k) -> p t k", k=6)
                for side, (inc_t, w_t) in enumerate(((incl, ws), (incn, wn))):
                    for r in range(3):
                        g = spool.tile([128, MEGA * 6], f16)
                        nc.vector.scalar_tensor_tensor(
                            out=g[:], in0=inc_t[:], scalar=float(r + 1), in1=w_t[:],
                            op0=alu.is_equal, op1=alu.mult)
                        g3 = g.rearrange("p (t j) -> p t j", j=6)
                        with nc.allow_low_precision(
                                reason="sum of <=1 nonzero small int, exact in fp16"):
                            nc.vector.tensor_reduce(
                                out=Ps3[:, :, 3 * side + r:3 * side + r + 1],
                                in_=g3[:], axis=mybir.AxisListType.X, op=alu.add)

                # ---- duplicate merge: values v0..v2, kill indices for dups ----
                Pv = spool.tile([128, MEGA * 6], f16)
                Pv3 = Pv.rearrange("p (t k) -> p t k", k=6)
                Pi = spool.tile([128, MEGA * 6], f16)
                Pi3 = Pi.rearrange("p (t k) -> p t k", k=6)
                for side in range(2):
                    o = 3 * side
                    s0 = Ps3[:, :, o:o + 1]
                    s1 = Ps3[:, :, o + 1:o + 2]
                    s2 = Ps3[:, :, o + 2:o + 3]
                    a = spool.tile([128, MEGA], f16)
                    a3 = a.rearrange("p (t k) -> p t k", k=1)
                    nc.vector.tensor_tensor(out=a3[:], in0=s1, in1=s0, op=alu.is_equal)
                    b = spool.tile([128, MEGA], f16)
                    b3 = b.rearrange("p (t k) -> p t k", k=1)
                    nc.vector.tensor_tensor(out=b3[:], in0=s2, in1=s0, op=alu.is_equal)
                    c = spool.tile([128, MEGA], f16)
                    c3 = c.rearrange("p (t k) -> p t k", k=1)
                    nc.vector.tensor_tensor(out=c3[:], in0=s2, in1=s1, op=alu.is_equal)
                    u = spool.tile([128, MEGA], f16)      # 1 - c
                    nc.vector.tensor_scalar(out=u[:], in0=c[:], scalar1=-1.0,
                                            scalar2=1.0, op0=alu.mult, op1=alu.add)
                    t1 = spool.tile([128, MEGA], f16)     # max(a, 1-c)
                    nc.vector.tensor_tensor(out=t1[:], in0=a[:], in1=u[:], op=alu.max)
                    t2 = spool.tile([128, MEGA], f16)     # b * t1
                    nc.vector.tensor_tensor(out=t2[:], in0=b[:], in1=t1[:],
                                            op=alu.mult)
                    t23 = t2.rearrange("p (t k) -> p t k", k=1)
                    # v0 = (a + 1) + t2
                    nc.vector.scalar_tensor_tensor(
                        out=Pv3[:, :, o:o + 1], in0=a3[:], scalar=1.0, in1=t23[:],
                        op0=alu.add, op1=alu.add)
                    r1 = spool.tile([128, MEGA], f16)     # 1 - a
                    nc.vector.tensor_scalar(out=r1[:], in0=a[:], scalar1=-1.0,
                                            scalar2=1.0, op0=alu.mult, op1=alu.add)
                    q = spool.tile([128, MEGA], f16)      # c * (1 - a)
                    nc.vector.tensor_tensor(out=q[:], in0=c[:], in1=r1[:],
                                            op=alu.mult)
                    r13 = r1.rearrange("p (t k) -> p t k", k=1)
                    q3 = q.rearrange("p (t k) -> p t k", k=1)
                    nc.vector.tensor_tensor(out=Pv3[:, :, o + 1:o + 2], in0=r13[:],
                                            in1=q3[:], op=alu.add)
                    ub = spool.tile([128, MEGA], f16)     # 1 - b
                    nc.vector.tensor_scalar(out=ub[:], in0=b[:], scalar1=-1.0,
                                            scalar2=1.0, op0=alu.mult, op1=alu.add)
                    ub3 = ub.rearrange("p (t k) -> p t k", k=1)
                    u3 = u.rearrange("p (t k) -> p t k", k=1)
                    nc.vector.tensor_tensor(out=Pv3[:, :, o + 2:o + 3], in0=ub3[:],
                                            in1=u3[:], op=alu.mult)
                    k2 = spool.tile([128, MEGA], f16)     # max(b, c): slot2 killed
                    nc.vector.tensor_tensor(out=k2[:], in0=b[:], in1=c[:], op=alu.max)
                    k23 = k2.rearrange("p (t k) -> p t k", k=1)
                    # raw indices (pre-offset); killed slots pushed very negative
                    nc.vector.tensor_copy(out=Pi3[:, :, o:o + 1], in_=s0)
                    nc.vector.scalar_tensor_tensor(
                        out=Pi3[:, :, o + 1:o + 2], in0=a3[:], scalar=KILL, in1=s1,
                        op0=alu.mult, op1=alu.add)
                    nc.vector.scalar_tensor_tensor(
                        out=Pi3[:, :, o + 2:o + 3], in0=k23[:], scalar=KILL, in1=s2,
                        op0=alu.mult, op1=alu.add)

                Pif = spool.tile([128, MEGA * 6], f16)
                nc.vector.tensor_tensor(out=Pif[:], in0=Pi[:], in1=co[:], op=alu.add)
                Pidx = spool.tile([128, MEGA * 6], i16)
                nc.vector.tensor_copy(out=Pidx[:], in_=Pif[:])
                Pidx3 = Pidx.rearrange("p (t k) -> p t k", k=6)

                # ================= per-superblock main pipeline ===============
                for sl in range(SB_PER_MEGA):
                    sb = mg * SB_PER_MEGA + sl
                    # one-hot scatter: A = (128, SBT * [stm 128 | nstm 128])
                    A = mpool.tile([128, SBT * 256], f16)
                    nc.gpsimd.local_scatter(
                        out_ap=A[:],
                        data_ap=Pv3[:, SBT * sl:SBT * (sl + 1), :],
                        idxs_ap=Pidx3[:, SBT * sl:SBT * (sl + 1), :],
                        channels=128, num_elems=SBT * 256, num_idxs=SBT * 6)

                    ATs = mpool.tile([128, BT], f16)
                    ATn = mpool.tile([128, BT], f16)
                    for tl in range(SBT):
                        nc.sync.dma_start_transpose(
                            out=ATs[:, 128 * tl:128 * (tl + 1)],
                            in_=A[:, 256 * tl:256 * tl + 128])
                        nc.sync.dma_start_transpose(
                            out=ATn[:, 128 * tl:128 * (tl + 1)],
                            in_=A[:, 256 * tl + 128:256 * tl + 256])

                    # FT embedding matmuls (K=128 incl. 8 zero rows, M=128)
                    accs = []
                    for si, AT in enumerate((ATs, ATn)):
                        for ch in range(2):
                            acc = ppool.tile([128, BT], mybir.dt.float32,
                                             name=f"acc{2 * si + ch}")
                            nc.tensor.matmul(out=acc[:],
                                             lhsT=wft[:, 128 * ch:128 * (ch + 1)],
                                             rhs=AT[:])
                            accs.append(acc)

                    # relu(acc + ft_b) -> fp16 SBUF; split DVE / ACT
                    xs = []
                    for ci, acc in enumerate(accs):
                        x = mpool.tile([128, BT], f16, name=f"x{ci}")
                        bcol = ftb[:, (ci % 2):(ci % 2) + 1]
                        if ci < 2:
                            nc.vector.tensor_scalar(out=x[:], in0=acc[:],
                                                    scalar1=bcol, scalar2=0.0,
                                                    op0=alu.add, op1=alu.max)
                        else:
                            nc.scalar.activation(out=x[:], in_=acc[:], func=AF.Relu,
                                                 bias=bcol, scale=1.0)
                        xs.append(x)

                    dsb = mpool.tile([DENSE + 1, BT], f16)
                    nc.sync.dma_start(out=dsb[:],
                                      in_=ap_de[:, BT * sb:BT * (sb + 1)])

                    # W1: dense chunk first (writes const row, start=True)
                    h_ps = ppool.tile([HID + 1, BT], mybir.dt.float32, bufs=2)
                    nc.tensor.matmul(out=h_ps[:], lhsT=w1d[:], rhs=dsb[:],
                                     start=True, stop=False)
                    for ch in range(4):
                        nc.tensor.matmul(out=h_ps[0:HID, :],
                                         lhsT=w1c[:, 32 * ch:32 * (ch + 1)],
                                         rhs=xs[ch][:],
                                         start=False, stop=(ch == 3))
                    h_sb = mpool.tile([HID + 1, BT], f16)
                    nc.scalar.activation(out=h_sb[:], in_=h_ps[:], func=AF.Relu)

                    # W2 (policy 60 rows + value row 60)
                    pol_ps = ppool.tile([HEX + 1, BT], mybir.dt.float32, bufs=2)
                    nc.tensor.matmul(out=pol_ps[:], lhsT=w2[:], rhs=h_sb[:])
                    pol_sb = mpool.tile([HEX + 1, BT], mybir.dt.float32)
                    nc.vector.tensor_copy(out=pol_sb[:, 0:BT // 2],
                                          in_=pol_ps[:, 0:BT // 2])
                    nc.scalar.copy(out=pol_sb[:, BT // 2:BT],
                                   in_=pol_ps[:, BT // 2:BT])
                    nc.sync.dma_start(out=ap_pol[:, BT * sb:BT * (sb + 1)],
                                      in_=pol_sb[0:HEX, :])
                    nc.sync.dma_start(out=vstage[sb:sb + 1, :],
                                      in_=pol_sb[HEX:HEX + 1, :])

            # ---- value: tanh over all staged pre-activations at once ----
            vt = mpool.tile([NSB, BT], mybir.dt.float32)
            nc.sync.dma_start(out=vt[:], in_=vstage[:])
            vo = mpool.tile([NSB, BT], mybir.dt.float32)
            nc.scalar.activation(out=vo[:], in_=vt[:], func=AF.Tanh)
            nc.sync.dma_start(out=ap_val[:], in_=vo[:])

    nc.compile()
    return nc


def _stage_consts():
    t = np.arange(MEGA)
    j = np.arange(6)
    cj = np.broadcast_to((j + 1).astype(np.float16), (128, MEGA, 6))
    cr = np.broadcast_to((j > 0).astype(np.float16), (128, MEGA, 6))
    co = (256 * (t[:, None] % SBT) + 128 * (j[None, :] >= 3)).astype(np.float16)
    co = np.broadcast_to(co, (128, MEGA, 6))
    return (np.ascontiguousarray(cj).reshape(128, MEGA * 6),
            np.ascontiguousarray(cr).reshape(128, MEGA * 6),
            np.ascontiguousarray(co).reshape(128, MEGA * 6))


def _stage_weights(ft_w, ft_b, W1, b1, W2v, b2v, W2p, b2p):
    w_ft = np.zeros((128, FT_DIM), np.float16)
    w_ft[:PIECE] = ft_w.astype(np.float16)
    w_ftb = np.zeros((128, 2), np.float32)
    w_ftb[:FT_DIM // 2, 0] = ft_b[:FT_DIM // 2]
    w_ftb[:FT_DIM // 2, 1] = ft_b[FT_DIM // 2:]
    # w1c[:, 32c:32c+32] must be W1[:, 128c:128c+128].T
    w1c = np.zeros((128, 128), np.float16)
    for ch in range(4):
        w1c[:, 32 * ch:32 * (ch + 1)] = W1[:, 128 * ch:128 * (ch + 1)].T
    w1d = np.zeros((DENSE + 1, HID + 1), np.float16)
    w1d[:DENSE, :HID] = W1[:, 512:].T
    w1d[DENSE, :HID] = b1
    w1d[DENSE, HID] = 1.0            # h const row source
    w2 = np.zeros((HID + 1, HEX + 1), np.float16)
    w2[:HID, :HEX] = W2p.T
    w2[:HID, HEX] = W2v[0]
    w2[HID, :HEX] = b2p
    w2[HID, HEX] = b2v[0]
    return w_ft, w_ftb, w1c, w1d, w2


def kernel(sparse_batch, dense_batch, stm_players, ft_w, ft_b, W1, b1,
           W2v, b2v, W2p, b2p):
    from concourse.bass_utils import run_bass_kernel_spmd

    if "nc" not in _CACHE:
        _CACHE["nc"] = _build_nc()
    nc = _CACHE["nc"]

    cj, cr, co = _stage_consts()
    w_ft, w_ftb, w1c, w1d, w2 = _stage_weights(
        np.asarray(ft_w, np.float32), np.asarray(ft_b, np.float32),
        np.asarray(W1, np.float32), np.asarray(b1, np.float32),
        np.asarray(W2v, np.float32), np.asarray(b2v, np.float32),
        np.asarray(W2p, np.float32), np.asarray(b2p, np.float32))

    sparse = np.asarray(sparse_batch)
    stm = np.asarray(stm_players)
    dense = np.asarray(dense_batch, np.float32)

    in_maps = []
    for c in range(N_CORES):
        lo, hi = c * BS, (c + 1) * BS
        sp = sparse[lo:hi].astype(np.float16)            # values < 120, exact
        sp = sp.reshape(NMEGA, MEGA, 128, 6).transpose(0, 2, 1, 3)
        sp = np.ascontiguousarray(sp).reshape(NMEGA, 128, MEGA * 6)
        st = (stm[lo:hi] == 0).astype(np.float16)        # stm0 indicator
        st = np.repeat(st.reshape(NMEGA, MEGA, 128, 1), 6, axis=3)
        st = st.transpose(0, 2, 1, 3)
        st = np.ascontiguousarray(st).reshape(NMEGA, 128, MEGA * 6)
        de = np.ones((DENSE + 1, BS), np.float16)
        de[:DENSE] = dense[lo:hi].T.astype(np.float16)
        in_maps.append({
            "x_sparse": sp, "x_stm6": st, "x_dense": de,
            "c_j": cj, "c_r": cr, "c_o": co,
            "w_ft": w_ft, "w_ftb": w_ftb, "w_w1c": w1c, "w_w1d": w1d,
            "w_w2": w2,
        })

    import os
    trace = bool(os.environ.get("KBENCH_TRACE"))
    res = run_bass_kernel_spmd(nc, in_maps, core_ids=list(range(N_CORES)),
                               trace=trace)
    _CACHE["last_results"] = res

    policy = np.empty((B_TOTAL, HEX), np.float32)
    value = np.empty((B_TOTAL,), np.float32)
    for c in range(N_CORES):
        lo, hi = c * BS, (c + 1) * BS
        policy[lo:hi] = res.results[c]["policy_t"].T
        value[lo:hi] = res.results[c]["value_s"].reshape(-1)
    return policy, value


# revision 14
# speedup vs baseline: 421.7577x; 421.7577x over previous
"""NNUE-MCTS model kernel for 8 Trainium2 NeuronCores.

Strategy (pure data parallel: batch sharded 8 ways, same Bass program per core).

Per core, transposed pipeline (feature dims on SBUF partitions, batch on the
free dim), superblocks of 512 batch columns:

  1. Selection logic in natural layout (batch on partitions), vectorized over
     mega-tiles of 64x128 batch rows: one fused mask op, one segmented-cumsum
     tensor_tensor_scan, rank-select ops -> top-3 feature ids per side (rank r
     value = sum(X * [incl==r+1] * mask), which is 0 == the pad id when fewer
     than 3 hits) -> duplicate merge to (index, count): v0 = 1+a+b,
     v1 = (1+c)(1-a), v2 = 1-max(b,c); killed slots' indices pushed negative.
     Two constant slots per tile scatter 1.0 into row 120 of each block,
     which multiplies the ft_b row folded into the weight chunks.
  2. GPSIMD local_scatter builds A (128 batch, 8 blocks x 128) fp16 per
     superblock: blocks 0-3 stm tiles, 4-7 nstm tiles.
  3. ONE xbar dma transpose per superblock: A (128,1024) -> AT (128,8,128)
     with AT[s,b,p] = A[p,128b+s]; stm rhs = AT[:, :512], nstm = AT[:, 512:].
  4. FT embedding matmuls (ft_w fp16 chunks stationary incl. ft_b row 120);
     both chunks of a side accumulate into one (128,1024) 2-bank PSUM tile,
     moved+relu'd to SBUF in a single op (DVE for stm, ACT for nstm).
  5. W1 as 5 K-chunk matmuls (dense chunk staged with a const-1 row that also
     creates h's const row; b1 folded there), relu -> h (33, 512) fp16.
  6. W2p/W2v/b2 packed into one (33, 61) matmul; one full-width PSUM->SBUF
     copy per superblock alternating DVE/ACT into an fp16 staging tile;
     policy DMA'd out per 8 superblocks; value row staged to DRAM and tanh'd
     in one (64, 512) pass at the end.

All matmul operands fp16 (one-hot counts and indices exact; ~6e-4 relative
error end to end); PSUM accumulates fp32.
"""

import numpy as np

B_TOTAL = 262144
N_CORES = 8
BS = B_TOTAL // N_CORES          # 32768 rows per core
NT = BS // 128                   # 256 tiles of 128 rows
MEGA = 64                        # tiles per selection mega-block
NMEGA = NT // MEGA               # 4
SBT = 4                          # tiles per superblock
NSB = NT // SBT                  # 64 superblocks
BT = 128 * SBT                   # 512 batch columns per superblock
SB_PER_MEGA = MEGA // SBT        # 16
POLG = 8                         # superblocks per policy-staging group
NPOLG = NSB // POLG              # 8
NSLOT = 8                        # 6 selection slots + 2 const bias slots

FT_DIM = 256
PIECE = 120
P1_CUT = 60
DENSE = 66
HID = 32
HEX = 60
KILL = -8192.0                   # pushes a duplicate slot's index negative

_CACHE = {}


def _build_nc():
    import concourse.bacc as bacc
    import concourse.mybir as mybir
    from concourse.tile import TileContext

    dt = mybir.dt
    alu = mybir.AluOpType
    AF = mybir.ActivationFunctionType
    f16, f32, i16 = dt.float16, dt.float32, dt.int16

    nc = bacc.Bacc("TRN2", target_bir_lowering=False, debug=False)

    # ---- DRAM I/O (per-core shapes) ----
    d_sp = nc.dram_tensor("x_sparse", (NMEGA, 128, MEGA * 6), f16, kind="ExternalInput")
    d_st = nc.dram_tensor("x_stm6", (NMEGA, 128, MEGA * 6), f16, kind="ExternalInput")
    d_de = nc.dram_tensor("x_dense", (DENSE + 1, BS), f16, kind="ExternalInput")
    d_cj = nc.dram_tensor("c_j", (128, MEGA * 6), f16, kind="ExternalInput")
    d_cr = nc.dram_tensor("c_r", (128, MEGA * 6), f16, kind="ExternalInput")
    d_co = nc.dram_tensor("c_o", (128, MEGA * NSLOT), f16, kind="ExternalInput")
    d_wft = nc.dram_tensor("w_ft", (128, FT_DIM), f16, kind="ExternalInput")
    d_w1c = nc.dram_tensor("w_w1c", (128, 128), f16, kind="ExternalInput")
    d_w1d = nc.dram_tensor("w_w1d", (DENSE + 1, HID + 1), f16, kind="ExternalInput")
    d_w2 = nc.dram_tensor("w_w2", (HID + 1, HEX + 1), f16, kind="ExternalInput")
    d_pol = nc.dram_tensor("policy_t", (HEX, BS), f16, kind="ExternalOutput")
    d_val = nc.dram_tensor("value_s", (NSB, BT), f32, kind="ExternalOutput")

    ap_sp, ap_st, ap_de = d_sp.ap(), d_st.ap(), d_de.ap()
    ap_pol, ap_val = d_pol.ap(), d_val.ap()

    with TileContext(nc) as tc:
        with tc.tile_pool(name="const", bufs=1) as cpool, \
             tc.tile_pool(name="sel", bufs=2) as spool, \
             tc.tile_pool(name="mm", bufs=3) as mpool, \
             tc.tile_pool(name="stage", bufs=2) as gpool, \
             tc.tile_pool(name="dstage", bufs=1, space="DRAM") as dpool, \
             tc.tile_pool(name="ps", bufs=1, space="PSUM") as ppool:

            # ---- constants / weights into SBUF once ----
            wft = cpool.tile([128, FT_DIM], f16)
            nc.sync.dma_start(out=wft[:], in_=d_wft.ap()[:])
            w1c = cpool.tile([128, 128], f16)
            nc.sync.dma_start(out=w1c[:], in_=d_w1c.ap()[:])
            w1d = cpool.tile([DENSE + 1, HID + 1], f16)
            nc.sync.dma_start(out=w1d[:], in_=d_w1d.ap()[:])
            w2 = cpool.tile([HID + 1, HEX + 1], f16)
            nc.sync.dma_start(out=w2[:], in_=d_w2.ap()[:])
            cj = cpool.tile([128, MEGA * 6], f16)
            nc.sync.dma_start(out=cj[:], in_=d_cj.ap()[:])
            cr = cpool.tile([128, MEGA * 6], f16)
            nc.sync.dma_start(out=cr[:], in_=d_cr.ap()[:])
            co = cpool.tile([128, MEGA * NSLOT], f16)
            nc.sync.dma_start(out=co[:], in_=d_co.ap()[:])

            vstage = dpool.tile([NPOLG, POLG * BT], f16)

            for mg in range(NMEGA):
                # ================= selection (natural layout) =================
                X = spool.tile([128, MEGA * 6], f16)
                nc.sync.dma_start(out=X[:], in_=ap_sp[mg])
                T6 = spool.tile([128, MEGA * 6], f16)
                nc.sync.dma_start(out=T6[:], in_=ap_st[mg])
                dsb = gpool.tile([DENSE + 1, SB_PER_MEGA * BT], f16)
                nc.sync.dma_start(
                    out=dsb[:],
                    in_=ap_de[:, mg * SB_PER_MEGA * BT:(mg + 1) * SB_PER_MEGA * BT])

                msk = spool.tile([128, MEGA * 6], f16)
                nc.vector.scalar_tensor_tensor(
                    out=msk[:], in0=X[:], scalar=float(P1_CUT), in1=T6[:],
                    op0=alu.is_lt, op1=alu.is_equal)
                # segmented inclusive prefix count: state = cr*state + msk
                incl = spool.tile([128, MEGA * 6], f16)
                nc.vector.tensor_tensor_scan(out=incl[:], data0=cr[:], data1=msk[:],
                                             initial=0.0, op0=alu.mult, op1=alu.add)
                incn = spool.tile([128, MEGA * 6], f16)
                nc.vector.tensor_tensor(out=incn[:], in0=cj[:], in1=incl[:],
                                        op=alu.subtract)
                ws = spool.tile([128, MEGA * 6], f16)
                nc.vector.tensor_tensor(out=ws[:], in0=X[:], in1=msk[:], op=alu.mult)
                wn = spool.tile([128, MEGA * 6], f16)
                nc.vector.tensor_tensor(out=wn[:], in0=X[:], in1=ws[:],
                                        op=alu.subtract)

                # P tiles, viewed (128, MEGA, NSLOT): 0..2 stm, 3..5 nstm, 6..7 bias
                Ps = spool.tile([128, MEGA * NSLOT], f16)
                Ps3 = Ps.rearrange("p (t k) -> p t k", k=NSLOT)
                for side, (inc_t, w_t) in enumerate(((incl, ws), (incn, wn))):
                    for r in range(3):
                        g = spool.tile([128, MEGA * 6], f16)
                        nc.vector.scalar_tensor_tensor(
                            out=g[:], in0=inc_t[:], scalar=float(r + 1), in1=w_t[:],
                            op0=alu.is_equal, op1=alu.mult)
                        g3 = g.rearrange("p (t j) -> p t j", j=6)
                        with nc.allow_low_precision(
                                reason="sum of <=1 nonzero small int, exact in fp16"):
                            nc.vector.tensor_reduce(
                                out=Ps3[:, :, 3 * side + r:3 * side + r + 1],
                                in_=g3[:], axis=mybir.AxisListType.X, op=alu.add)

                # ---- duplicate merge ----
                Pv = spool.tile([128, MEGA * NSLOT], f16)
                Pv3 = Pv.rearrange("p (t k) -> p t k", k=NSLOT)
                Pi = spool.tile([128, MEGA * NSLOT], f16)
                Pi3 = Pi.rearrange("p (t k) -> p t k", k=NSLOT)
                # bias slots: index offset comes entirely from c_o; value 1.0
                nc.vector.memset(Pi3[:, :, 6:8], 0.0)
                nc.vector.memset(Pv3[:, :, 6:8], 1.0)
                for side in range(2):
                    o = 3 * side
                    s0 = Ps3[:, :, o:o + 1]
                    s1 = Ps3[:, :, o + 1:o + 2]
                    s2 = Ps3[:, :, o + 2:o + 3]
                    a = spool.tile([128, MEGA], f16)
                    a3 = a.rearrange("p (t k) -> p t k", k=1)
                    nc.vector.tensor_tensor(out=a3[:], in0=s1, in1=s0, op=alu.is_equal)
                    b = spool.tile([128, MEGA], f16)
                    b3 = b.rearrange("p (t k) -> p t k", k=1)
                    nc.vector.tensor_tensor(out=b3[:], in0=s2, in1=s0, op=alu.is_equal)
                    c = spool.tile([128, MEGA], f16)
                    c3 = c.rearrange("p (t k) -> p t k", k=1)
                    nc.vector.tensor_tensor(out=c3[:], in0=s2, in1=s1, op=alu.is_equal)
                    # v0 = (a + 1) + b   (b&c -> a, so "s2 merges to s0" == b)
                    nc.vector.scalar_tensor_tensor(
                        out=Pv3[:, :, o:o + 1], in0=a3[:], scalar=1.0, in1=b3[:],
                        op0=alu.add, op1=alu.add)
                    r1 = spool.tile([128, MEGA], f16)     # 1 - a
                    nc.vector.tensor_scalar(out=r1[:], in0=a[:], scalar1=-1.0,
                                            scalar2=1.0, op0=alu.mult, op1=alu.add)
                    r13 = r1.rearrange("p (t k) -> p t k", k=1)
                    # v1 = (c + 1) * (1 - a)
                    nc.vector.scalar_tensor_tensor(
                        out=Pv3[:, :, o + 1:o + 2], in0=c3[:], scalar=1.0, in1=r13[:],
                        op0=alu.add, op1=alu.mult)
                    k2 = spool.tile([128, MEGA], f16)     # max(b, c): slot2 killed
                    nc.vector.tensor_tensor(out=k2[:], in0=b[:], in1=c[:], op=alu.max)
                    k23 = k2.rearrange("p (t k) -> p t k", k=1)
                    # v2 = 1 - max(b, c)
                    nc.vector.tensor_scalar(out=Pv3[:, :, o + 2:o + 3], in0=k23[:],
                                            scalar1=-1.0, scalar2=1.0,
                                            op0=alu.mult, op1=alu.add)
                    # raw indices; killed slots pushed very negative
                    nc.vector.tensor_copy(out=Pi3[:, :, o:o + 1], in_=s0)
                    nc.vector.scalar_tensor_tensor(
                        out=Pi3[:, :, o + 1:o + 2], in0=a3[:], scalar=KILL, in1=s1,
                        op0=alu.mult, op1=alu.add)
                    nc.vector.scalar_tensor_tensor(
                        out=Pi3[:, :, o + 2:o + 3], in0=k23[:], scalar=KILL, in1=s2,
                        op0=alu.mult, op1=alu.add)

                Pif = spool.tile([128, MEGA * NSLOT], f16)
                nc.vector.tensor_tensor(out=Pif[:], in0=Pi[:], in1=co[:], op=alu.add)
                Pidx = spool.tile([128, MEGA * NSLOT], i16)
                nc.vector.tensor_copy(out=Pidx[:], in_=Pif[:])
                Pidx3 = Pidx.rearrange("p (t k) -> p t k", k=NSLOT)

                # ================= per-superblock main pipeline ===============
                for pr in range(SB_PER_MEGA // 2):
                    # scatter + one batched transpose per pair of superblocks
                    A = mpool.tile([128, 2 * SBT * 256], f16, name="A")
                    AT = mpool.tile([128, 2 * SBT * 256], f16, name="AT")
                    for hf in range(2):
                        sl = 2 * pr + hf
                        nc.gpsimd.local_scatter(
                            out_ap=A[:, hf * SBT * 256:(hf + 1) * SBT * 256],
                            data_ap=Pv3[:, SBT * sl:SBT * (sl + 1), :],
                            idxs_ap=Pidx3[:, SBT * sl:SBT * (sl + 1), :],
                            channels=128, num_elems=SBT * 256,
                            num_idxs=SBT * NSLOT)
                    AT3 = AT.rearrange("s (b p) -> s b p", p=128)
                    nc.sync.dma_start_transpose(out=AT3[:], in_=A[:])

                    for hf in range(2):
                        sl = 2 * pr + hf
                        sb = mg * SB_PER_MEGA + sl
                        if sb % POLG == 0:
                            pol_sb = gpool.tile([HEX + 1, POLG * BT], f16,
                                                name="pol_sb")
                        ps_off = (sb % POLG) * BT
                        half = hf * SBT * 256

                        # FT embedding: 2 chunks accumulate into one
                        # 2-bank PSUM tile per side
                        xs = []
                        for si in range(2):
                            rhs = AT[:, half + 512 * si:half + 512 * (si + 1)]
                            acc = ppool.tile([128, 2 * BT], mybir.dt.float32,
                                             name="acc", bufs=3)
                            for ch in range(2):
                                nc.tensor.matmul(
                                    out=acc[:, BT * ch:BT * (ch + 1)],
                                    lhsT=wft[:, 128 * ch:128 * (ch + 1)],
                                    rhs=rhs)
                            x = mpool.tile([128, 2 * BT], f16, name=f"x{si}")
                            if si == 0:
                                nc.vector.tensor_scalar(out=x[:], in0=acc[:],
                                                        scalar1=0.0, scalar2=None,
                                                        op0=alu.max)
                            else:
                                nc.scalar.activation(out=x[:], in_=acc[:],
                                                     func=AF.Relu)
                            xs.append(x)

                        # W1: dense chunk first (writes const row, start=True)
                        h_ps = ppool.tile([HID + 1, BT], mybir.dt.float32, bufs=1)
                        nc.tensor.matmul(out=h_ps[:], lhsT=w1d[:],
                                         rhs=dsb[:, sl * BT:(sl + 1) * BT],
                                         start=True, stop=False)
                        for ch in range(4):
                            nc.tensor.matmul(out=h_ps[0:HID, :],
                                             lhsT=w1c[:, 32 * ch:32 * (ch + 1)],
                                             rhs=xs[ch // 2][:, BT * (ch % 2):
                                                             BT * (ch % 2 + 1)],
                                             start=False, stop=(ch == 3))
                        h_sb = mpool.tile([HID + 1, BT], f16)
                        nc.scalar.activation(out=h_sb[:], in_=h_ps[:], func=AF.Relu)

                        # W2 (policy 60 rows + value row 60)
                        pol_ps = ppool.tile([HEX + 1, BT], mybir.dt.float32, bufs=1)
                        nc.tensor.matmul(out=pol_ps[:], lhsT=w2[:], rhs=h_sb[:])
                        if sb % 2 == 0:
                            nc.vector.tensor_copy(
                                out=pol_sb[:, ps_off:ps_off + BT], in_=pol_ps[:])
                        else:
                            nc.scalar.copy(
                                out=pol_sb[:, ps_off:ps_off + BT], in_=pol_ps[:])

                        if sb % POLG == POLG - 1:
                            g = sb // POLG
                            nc.sync.dma_start(
                                out=ap_pol[:, g * POLG * BT:(g + 1) * POLG * BT],
                                in_=pol_sb[0:HEX, :])
                            nc.sync.dma_start(out=vstage[g:g + 1, :],
                                              in_=pol_sb[HEX:HEX + 1, :])

            # ---- value: tanh over all staged pre-activations at once ----
            vt = mpool.tile([NSB, BT], f16)
            nc.sync.dma_start(
                out=vt[:], in_=vstage.rearrange("a (b c) -> (a b) c", c=BT))
            vo = mpool.tile([NSB, BT], mybir.dt.float32)
            nc.scalar.activation(out=vo[:], in_=vt[:], func=AF.Tanh)
            nc.sync.dma_start(out=ap_val[:], in_=vo[:])

    nc.compile()
    return nc


def _stage_consts():
    t = np.arange(MEGA)
    cj = np.broadcast_to((np.arange(6) + 1).astype(np.float16), (128, MEGA, 6))
    cr = np.broadcast_to((np.arange(6) > 0).astype(np.float16), (128, MEGA, 6))
    k = np.arange(NSLOT)
    co = (128 * (t[:, None] % SBT) + 512 * ((k[None, :] >= 3) & (k[None, :] < 6))
          + (k[None, :] == 6) * 120 + (k[None, :] == 7) * (512 + 120))
    co = np.broadcast_to(co.astype(np.float16), (128, MEGA, NSLOT))
    return (np.ascontiguousarray(cj).reshape(128, MEGA * 6),
            np.ascontiguousarray(cr).reshape(128, MEGA * 6),
            np.ascontiguousarray(co).reshape(128, MEGA * NSLOT))


def _stage_weights(ft_w, ft_b, W1, b1, W2v, b2v, W2p, b2p):
    w_ft = np.zeros((128, FT_DIM), np.float16)
    w_ft[:PIECE] = ft_w.astype(np.float16)
    w_ft[PIECE] = ft_b.astype(np.float16)     # multiplied by scattered 1.0s
    # w1c[:, 32c:32c+32] = W1[:, 128c:128c+128].T
    w1c = np.zeros((128, 128), np.float16)
    for ch in range(4):
        w1c[:, 32 * ch:32 * (ch + 1)] = W1[:, 128 * ch:128 * (ch + 1)].T
    w1d = np.zeros((DENSE + 1, HID + 1), np.float16)
    w1d[:DENSE, :HID] = W1[:, 512:].T
    w1d[DENSE, :HID] = b1
    w1d[DENSE, HID] = 1.0            # h const row source
    w2 = np.zeros((HID + 1, HEX + 1), np.float16)
    w2[:HID, :HEX] = W2p.T
    w2[:HID, HEX] = W2v[0]
    w2[HID, :HEX] = b2p
    w2[HID, HEX] = b2v[0]
    return w_ft, w1c, w1d, w2


def _stage_core_inputs(sparse, stm, dense, lo, hi):
    sp = sparse[lo:hi].astype(np.float16)            # values < 120, exact
    sp = sp.reshape(NMEGA, MEGA, 128, 6).transpose(0, 2, 1, 3)
    sp = np.ascontiguousarray(sp).reshape(NMEGA, 128, MEGA * 6)
    st = (stm[lo:hi] == 0).astype(np.float16)        # stm0 indicator
    st = np.repeat(st.reshape(NMEGA, MEGA, 128, 1), 6, axis=3)
    st = st.transpose(0, 2, 1, 3)
    st = np.ascontiguousarray(st).reshape(NMEGA, 128, MEGA * 6)
    de = np.ones((DENSE + 1, BS), np.float16)
    de[:DENSE] = dense[lo:hi].T.astype(np.float16)
    return sp, st, de


def kernel(sparse_batch, dense_batch, stm_players, ft_w, ft_b, W1, b1,
           W2v, b2v, W2p, b2p):
    import os
    from concourse.bass_utils import run_bass_kernel_spmd

    if "nc" not in _CACHE:
        _CACHE["nc"] = _build_nc()
    nc = _CACHE["nc"]

    cj, cr, co = _stage_consts()
    w_ft, w1c, w1d, w2 = _stage_weights(
        np.asarray(ft_w, np.float32), np.asarray(ft_b, np.float32),
        np.asarray(W1, np.float32), np.asarray(b1, np.float32),
        np.asarray(W2v, np.float32), np.asarray(b2v, np.float32),
        np.asarray(W2p, np.float32), np.asarray(b2p, np.float32))

    sparse = np.asarray(sparse_batch)
    stm = np.asarray(stm_players)
    dense = np.asarray(dense_batch, np.float32)

    in_maps = []
    for c in range(N_CORES):
        sp, st, de = _stage_core_inputs(sparse, stm, dense, c * BS, (c + 1) * BS)
        in_maps.append({
            "x_sparse": sp, "x_stm6": st, "x_dense": de,
            "c_j": cj, "c_r": cr, "c_o": co,
            "w_ft": w_ft, "w_w1c": w1c, "w_w1d": w1d, "w_w2": w2,
        })

    trace = bool(os.environ.get("KBENCH_TRACE"))
    res = run_bass_kernel_spmd(nc, in_maps, core_ids=list(range(N_CORES)),
                               trace=trace)
    _CACHE["last_results"] = res

    policy = np.empty((B_TOTAL, HEX), np.float32)
    value = np.empty((B_TOTAL,), np.float32)
    for c in range(N_CORES):
        lo, hi = c * BS, (c + 1) * BS
        policy[lo:hi] = res.results[c]["policy_t"].T.astype(np.float32)
        value[lo:hi] = res.results[c]["value_s"].reshape(-1)
    return policy, value
